# revision 18
# baseline (speedup 1.0000x reference)
# Trainium2 Bass kernel for nn_CrossAttentionLayer (linear attention with
# elu+1 feature map).
#
# Math (per batch n):
#   q = guidance @ Wq.T + bq ; k = x @ Wk.T + bk ; v = x @ Wv.T + bv
#   Q = elu(q)+1 ; K = elu(k)+1          (per head h, head dim D=64)
#   KV_h = K_h^T @ (v_h/S);  Z = 1/(Q_h . sum_s K_h + eps)
#   out_h = (Q_h @ KV_h) * Z * S         (the /S and *S cancel exactly)
#
# Sharding: 8 cores = batch(4) x {head-half for K/V, guidance-half for Q}.
# Core (b, j) computes K/V features for heads 4j..4j+3 over the FULL source
# sequence (so its KV/Ksum partial sums are exact), and the Q side for its
# 2048 guidance rows.  The two cores of a batch AllGather their KV/Ksum
# halves (bf16, 66KB) instead of AllReducing partial sums.
#
# All matmul operands are bf16 (1 PE cycle/row at ANY free size, unlike
# fp32r which needs >=256).  The host pre-transposes x/guidance to
# feature-major and pre-casts everything to bf16, which kills all on-chip
# PE transposes and halves HBM traffic.  f32 is kept for PSUM accumulation
# and the final output.
#
# elu(t)+1 == max(min(exp(t), 1), t+1), computed as:
#   ACT: e = exp(psum);  Pool: u = max(psum+1, 0);  DVE: min(e,1), max(.,u)
#
# The per-head KV (64x64 diag blocks) and Ksum are packed into a single
# block-diag operand kvbd[128, pair, 130]: cols 0..127 = 2-head KV blocks,
# col 128/129 = per-head Ksum half-columns.  One matmul per head-pair then
# yields both the output features AND the denominators (cols 128/129).

import sys

import numpy as np

if "/opt/trn_rl_repo" not in sys.path:
    sys.path.insert(0, "/opt/trn_rl_repo")

import concourse.bacc as bacc
import concourse.mybir as mybir
import concourse.tile as tile
from concourse import bass_utils

P = 128
S = 4096
LC = 2048  # guidance rows per core
C = 512
CH = 256  # k/v output features per core (head half)
H = 8
D = 64
NCI = C // P  # 4 cin tiles
NST = S // P  # 32 s-tiles (full S per core)
EPS = 1e-6  # negligible vs denominators ~1e5; folded away

F32 = mybir.dt.float32
BF16 = mybir.dt.bfloat16

Exp = mybir.ActivationFunctionType.Exp
Relu = mybir.ActivationFunctionType.Relu
Copy = mybir.ActivationFunctionType.Copy
Add = mybir.AluOpType.add
Max = mybir.AluOpType.max
Mult = mybir.AluOpType.mult
Bypass = mybir.AluOpType.bypass

REPLICA_GROUPS = [[0, 1], [2, 3], [4, 5], [6, 7]]


def _build_nc(reps=1, with_bias=False):
    nc = bacc.Bacc(
        "TRN2",
        target_bir_lowering=False,
        debug=False,
        enable_asserts=False,
        num_devices=8,
    )
    xt = nc.dram_tensor("xt", [C, S], BF16, kind="ExternalInput").ap()
    gt = nc.dram_tensor("gt", [C, LC], BF16, kind="ExternalInput").ap()
    # k and v projection weights concatenated: one matmul set, one PSUM bank
    wkvt = nc.dram_tensor("wkvt", [C, C], BF16, kind="ExternalInput").ap()
    wqt = nc.dram_tensor("wqt", [C, C], BF16, kind="ExternalInput").ap()
    bkv = nc.dram_tensor("bkv", [1, C], BF16, kind="ExternalInput").ap()
    bq = nc.dram_tensor("bq", [C], F32, kind="ExternalInput").ap()
    outb = nc.dram_tensor("outb", [LC, C], F32, kind="ExternalOutput").ap()

    with tile.TileContext(nc) as tc:
        with tc.tile_pool(name="wpool", bufs=1) as wp:
            # weights resident in SBUF across reps
            wkv_sb = wp.tile([P, NCI, C], BF16)
            wq_sb = wp.tile([P, NCI, C], BF16)
            nc.sync.dma_start(wkv_sb, wkvt.rearrange("(t p) n -> p t n", p=P))
            nc.sync.dma_start(wq_sb, wqt.rearrange("(t p) n -> p t n", p=P))
            consts = dict(wkv=wkv_sb, wq=wq_sb)
            if with_bias:
                ones_row = wp.tile([1, P], BF16)
                nc.vector.memset(ones_row, 1.0)
                bkv_row = wp.tile([1, C], BF16)
                nc.sync.dma_start(bkv_row, bkv)
                bqT = wp.tile([P, NCI], F32)
                nc.sync.dma_start(bqT, bq.rearrange("(t p) -> p t", p=P))
                bqT1 = wp.tile([P, NCI], F32)
                nc.vector.tensor_scalar_add(bqT1, bqT, 1.0)
                consts.update(
                    ones_row=ones_row, bkv_row=bkv_row, bqT=bqT, bqT1=bqT1,
                )
            for rep in range(reps):
                _emit(nc, tc, consts, xt, gt, outb, rep=rep, with_bias=with_bias)

    nc.compile()
    return nc


def _emit(nc, tc, consts, xt, gt, outb, rep=0, with_bias=False):
    mm = nc.tensor.matmul
    wkv_sb, wq_sb = consts["wkv"], consts["wq"]
    with (
        tc.tile_pool(name=f"pp{rep}", bufs=1) as pp,
    ):
        # streamed inputs: x feature-major (full C), guidance feature-major
        xT = pp.tile([P, NCI, S], BF16)
        xt_r = xt.rearrange("(t p) s -> p t s", p=P)
        for c in range(8):
            sl = slice(c * 512, (c + 1) * 512)
            nc.gpsimd.dma_start(xT[:, :, sl], xt_r[:, :, sl])
        gT = pp.tile([P, NCI, LC], BF16)
        gt_r = gt.rearrange("(t p) s -> p t s", p=P)
        for c in range(2):
            sl = slice(c * 1024, (c + 1) * 1024)
            nc.gpsimd.dma_start(gT[:, :, sl], gt_r[:, :, sl])

        kvbd = pp.tile([P, 4, 130], BF16)
        stg = pp.tile([P, 260], BF16)
        stg2 = pp.tile([P, 2, 260], BF16)

        # ---------------- phase 1: x -> K,V -> KV, Ksum ----------------
        with (
            tc.tile_pool(name=f"p1_{rep}", bufs=3) as p1,
            tc.tile_pool(name=f"p1ps_{rep}", bufs=2, space="PSUM") as p1ps,
            tc.tile_pool(name=f"kvps_{rep}", bufs=1, space="PSUM") as kvps,
        ):
            kv_ps = kvps.tile([P, 2, 130], F32)

            def consume(pkv, st):
                pk = pkv[:, 0:CH]
                pv_r = pkv[:, CH:C].rearrange("p (a n) -> p a n", a=2)
                # K = elu(k)+1 = max(min(exp(k),1), k+1); the relu around
                # k+1 is redundant because min(exp(k),1) >= 0.
                e = p1.tile([P, CH], BF16, tag="e")
                nc.scalar.activation(e, pk, Exp)
                u = p1.tile([P, CH], BF16, tag="u")
                m = p1.tile([P, CH], BF16, tag="m")
                v_ext = p1.tile([P, 2, 130], BF16, tag="v")
                # u (k+1, from PSUM) and the V bf16 cast alternate between
                # ACT and DVE to balance the two queues
                if st % 2 == 0:
                    nc.scalar.activation(u, pk, Copy, bias=1.0)
                    nc.vector.tensor_copy(v_ext[:, :, 0:128], pv_r)
                else:
                    nc.vector.tensor_scalar_add(u, pk, 1.0)
                    nc.scalar.copy(v_ext[:, :, 0:128], pv_r)
                nc.vector.tensor_scalar_min(m, e, 1.0)
                k_sb = p1.tile([P, CH], BF16, tag="k")
                nc.vector.tensor_tensor(k_sb, m, u, Max)
                nc.vector.memset(v_ext[:, :, 128:130], 1.0)
                for j in range(2):
                    mm(kv_ps[:, j, :], k_sb[:, j * P : (j + 1) * P],
                       v_ext[:, j, :],
                       start=(st == 0 and j == 0),
                       stop=(st == NST - 1 and j == 1))

            prev = None
            for st in range(NST):
                ssl = slice(st * P, (st + 1) * P)
                pkv = p1ps.tile([P, C], F32, tag="pkv")
                if with_bias:
                    mm(pkv, consts["ones_row"], consts["bkv_row"],
                       start=True, stop=False)
                for ci in range(NCI):
                    mm(pkv, xT[:, ci, ssl], wkv_sb[:, ci, :],
                       start=(ci == 0 and not with_bias), stop=(ci == NCI - 1))
                # consume previous s-tile so PE never waits on the elu chain
                if prev is not None:
                    consume(prev, st - 1)
                prev = pkv
            consume(prev, NST - 1)

            # pack local pairs, AllGather both halves (concat = pairs 0..3)
            nc.vector.tensor_copy(stg.rearrange("p (a n) -> p a n", a=2), kv_ps)
            ccin = nc.dram_tensor(f"ccin{rep}", [P, 260], BF16).ap()
            ccout = nc.dram_tensor(f"ccout{rep}", [2 * P, 260], BF16).ap()
            nc.sync.dma_start(ccin, stg)
            nc.gpsimd.collective_compute(
                "AllGather",
                Bypass,
                replica_groups=REPLICA_GROUPS,
                ins=[ccin],
                outs=[ccout],
            )
            nc.sync.dma_start(stg2, ccout.rearrange("(g p) n -> p g n", p=P))
            # block-diag KV; cols 128/129 hold the per-head Ksum
            # half-columns so the output matmul also yields denominators
            nc.vector.memset(kvbd, 0.0)
            for t in range(4):
                src = stg2[:, t // 2, (t % 2) * 130 : (t % 2) * 130 + 130]
                nc.vector.tensor_copy(kvbd[0:D, t, 0:D], src[0:D, 0:D])
                nc.vector.tensor_copy(kvbd[D:P, t, D:2 * D], src[D:P, D:2 * D])
                nc.vector.tensor_copy(kvbd[0:D, t, 128:129], src[0:D, 128:129])
                nc.vector.tensor_copy(kvbd[D:P, t, 129:130], src[D:P, 128:129])

        # ---------------- phase 2: guidance -> Q -> out ----------------
        with (
            tc.tile_pool(name=f"p2_{rep}", bufs=2) as p2,
            tc.tile_pool(name=f"qts_{rep}", bufs=1) as qts,
            tc.tile_pool(name=f"p2ps_{rep}", bufs=3, space="PSUM") as p2ps,
            tc.tile_pool(name=f"pops_{rep}", bufs=2, space="PSUM") as pops,
        ):
            qTs = [qts.tile([P, NCI, C], BF16, name=f"qT{c}") for c in range(4)]

            def qproj(c):
                csl = slice(c * C, (c + 1) * C)
                for ct in range(NCI):
                    pq = p2ps.tile([P, C], F32, tag="pq")
                    for ci in range(NCI):
                        mm(pq, wq_sb[:, ci, ct * P : (ct + 1) * P],
                           gT[:, ci, csl],
                           start=(ci == 0), stop=(ci == NCI - 1))
                    e2 = p2.tile([P, C], BF16, tag="e2")
                    u2 = p2.tile([P, C], BF16, tag="u2")
                    if with_bias:
                        nc.scalar.activation(
                            e2, pq, Exp, bias=consts["bqT"][:, ct : ct + 1]
                        )
                        nc.scalar.activation(
                            u2, pq, Relu, bias=consts["bqT1"][:, ct : ct + 1]
                        )
                    else:
                        nc.scalar.activation(e2, pq, Exp)
                        nc.scalar.activation(u2, pq, Copy, bias=1.0)
                    m2 = p2.tile([P, C], BF16, tag="m2")
                    nc.vector.tensor_scalar_min(m2, e2, 1.0)
                    nc.vector.tensor_tensor(qTs[c][:, ct, :], m2, u2, Max)

            def tails(c):
                osb = p2.tile([P, 4, C], F32, tag="osb")
                for lt in range(4):
                    lsl = slice(lt * P, (lt + 1) * P)
                    po_a = pops.tile([P, 2, 130], F32, tag="poa")
                    po_b = pops.tile([P, 2, 130], F32, tag="pob")
                    for t in range(4):
                        tgt = po_a if t < 2 else po_b
                        mm(tgt[:, t % 2, :], qTs[c][:, t, lsl], kvbd[:, t, :],
                           start=(t % 2 == 0), stop=(t % 2 == 1))
                    zr = p2.tile([P, H], F32, tag="zr")
                    nc.vector.reciprocal(
                        zr[:, 0:4].rearrange("p (a h) -> p a h", a=2),
                        po_a[:, :, 128:130],
                    )
                    nc.vector.reciprocal(
                        zr[:, 4:8].rearrange("p (a h) -> p a h", a=2),
                        po_b[:, :, 128:130],
                    )
                    for half, po in ((0, po_a), (1, po_b)):
                        nc.vector.tensor_tensor(
                            osb[:, lt, half * 256 : (half + 1) * 256].rearrange(
                                "p (a h v) -> p a h v", a=2, h=2
                            ),
                            po[:, :, 0:128].rearrange("p a (h v) -> p a h v", h=2),
                            zr[:, half * 4 : (half + 1) * 4]
                            .rearrange("p (a h) -> p a h", a=2)[:, :, :, None]
                            .to_broadcast([P, 2, 2, D]),
                            Mult,
                        )
                nc.sync.dma_start(
                    outb[c * C : (c + 1) * C, :].rearrange(
                        "(lt p) n -> p lt n", p=P
                    ),
                    osb,
                )

            qproj(0)
            qproj(1)
            tails(0)
            qproj(2)
            tails(1)
            qproj(3)
            tails(2)
            tails(3)


_CACHE = {}


def _get_nc(reps=1, with_bias=False):
    key = ("nc", reps, with_bias)
    if key not in _CACHE:
        _CACHE[key] = _build_nc(reps, with_bias)
    return _CACHE[key]


def _make_runner(nc):
    """Build a reusable jitted SPMD runner for `nc` (mirrors
    bass2jax.run_bass_via_pjrt's multi-core branch, but caches the jit so
    repeated calls don't re-lower/re-compile)."""
    import jax
    from jax.sharding import Mesh, PartitionSpec
    from jax.experimental.shard_map import shard_map

    import concourse.mybir as mb
    from concourse import bass2jax

    bass2jax.install_neuronx_cc_hook()

    n_cores = 8
    partition_name = (
        nc.partition_id_tensor.name if nc.partition_id_tensor else None
    )
    in_names, out_names, out_avals, zero_shapes = [], [], [], []
    for alloc in nc.m.functions[0].allocations:
        if not isinstance(alloc, mb.MemoryLocationSet):
            continue
        name = alloc.memorylocations[0].name
        if alloc.kind == "ExternalInput":
            if name != partition_name:
                in_names.append(name)
        elif alloc.kind == "ExternalOutput":
            shape = tuple(alloc.tensor_shape)
            dtype = mb.dt.np(alloc.dtype)
            out_names.append(name)
            out_avals.append(jax.core.ShapedArray(shape, dtype))
            zero_shapes.append((shape, dtype))
    n_params = len(in_names)
    n_outs = len(out_names)
    all_names = in_names + out_names
    if partition_name is not None:
        all_names.append(partition_name)
    donate = tuple(range(n_params, n_params + n_outs))

    def _body(*args):
        operands = list(args)
        if partition_name is not None:
            operands.append(bass2jax.partition_id_tensor())
        outs = bass2jax._bass_exec_p.bind(
            *operands,
            out_avals=tuple(out_avals),
            in_names=tuple(all_names),
            out_names=tuple(out_names),
            lowering_input_output_aliases=(),
            sim_require_finite=True,
            sim_require_nnan=True,
            nc=nc,
        )
        return tuple(outs)

    devices = jax.devices()[:n_cores]
    mesh = Mesh(np.asarray(devices), ("core",))
    in_specs = (PartitionSpec("core"),) * (n_params + n_outs)
    out_specs = (PartitionSpec("core"),) * n_outs
    sharded = jax.jit(
        shard_map(
            _body, mesh=mesh, in_specs=in_specs, out_specs=out_specs,
            check_rep=False,
        ),
        donate_argnums=donate,
        keep_unused=True,
    )

    def _zeros():
        return [
            np.zeros((n_cores * sh[0], *sh[1:]), dt) for sh, dt in zero_shapes
        ]

    def runner(concat_in):
        out_arrs = sharded(*concat_in, *_zeros())
        return [
            {
                name: np.asarray(out_arrs[i]).reshape(
                    n_cores, *out_avals[i].shape
                )[c]
                for i, name in enumerate(out_names)
            }
            for c in range(n_cores)
        ]

    def concat(maps):
        return [
            np.concatenate([np.asarray(m[name]) for m in maps], axis=0)
            for name in in_names
        ]

    def timed(concat_in, n=10, warmup=2):
        """Time `n` executions with device-resident inputs and on-device
        donated zero outputs, so per-call host traffic is ~zero."""
        import time as _time
        import jax.numpy as jnp
        from jax.sharding import NamedSharding

        sh = NamedSharding(mesh, PartitionSpec("core"))
        dev_in = [jax.device_put(a, sh) for a in concat_in]

        @jax.jit
        def _mkzeros():
            return tuple(
                jnp.zeros((n_cores * s[0], *s[1:]), d) for s, d in zero_shapes
            )

        _mkzeros = jax.jit(_mkzeros, out_shardings=(sh,) * n_outs)
        times = []
        for i in range(warmup + n):
            z = jax.block_until_ready(_mkzeros())
            t0 = _time.perf_counter()
            outs = sharded(*dev_in, *z)
            jax.block_until_ready(outs)
            dt = _time.perf_counter() - t0
            if i >= warmup:
                times.append(dt)
        return times

    return runner, concat, timed


def _in_maps(x, guidance, Wq, bq, Wk, bk, Wv, bv):
    import ml_dtypes

    bf16 = ml_dtypes.bfloat16
    x = np.asarray(x, dtype=np.float32)
    guidance = np.asarray(guidance, dtype=np.float32)
    wqt = np.ascontiguousarray(np.asarray(Wq, dtype=np.float32).T.astype(bf16))
    wkt = np.asarray(Wk, dtype=np.float32).T.astype(bf16)
    wvt = np.asarray(Wv, dtype=np.float32).T.astype(bf16)
    bq = np.ascontiguousarray(bq, dtype=np.float32)
    bk = np.asarray(bk, dtype=np.float32).astype(bf16)
    bv = np.asarray(bv, dtype=np.float32).astype(bf16)
    maps = []
    for core in range(8):
        b, j = core // 2, core % 2
        csl = slice(j * CH, (j + 1) * CH)
        maps.append(
            {
                "xt": np.ascontiguousarray(x[b].T.astype(bf16)),
                "gt": np.ascontiguousarray(
                    guidance[b, j * LC : (j + 1) * LC].T.astype(bf16)
                ),
                "wqt": wqt,
                "wkvt": np.ascontiguousarray(
                    np.concatenate([wkt[:, csl], wvt[:, csl]], axis=1)
                ),
                "bq": bq,
                "bkv": np.ascontiguousarray(
                    np.concatenate([bk[csl], bv[csl]])
                ).reshape(1, C),
            }
        )
    return maps


def _gather(results):
    B = 4
    out = np.empty((B, 2 * LC, C), dtype=np.float32)
    for core in range(8):
        b, half = core // 2, core % 2
        out[b, half * LC : (half + 1) * LC] = results[core]["outb"]
    return out


def run(inputs, reps=1):
    with_bias = bool(
        np.any(inputs["bq"]) or np.any(inputs["bk"]) or np.any(inputs["bv"])
    )
    nc = _get_nc(reps, with_bias)
    key = ("runner", reps, with_bias)
    if key not in _CACHE:
        _CACHE[key] = _make_runner(nc)
    runner, concat, timed = _CACHE[key]
    maps = _in_maps(**inputs)
    return runner, timed, concat(maps)


def kernel(**inputs):
    runner, _, concat_in = run(inputs)
    return _gather(runner(concat_in))
